# revision 5
# baseline (speedup 1.0000x reference)
"""Trainium2 Bass kernel for nn_CrossfusionBidirectional.

Sharding: 8 cores = (batch b in {0,1}) x (query-row quarter qi in {0..3}).
Each core computes output rows [qi*784, (qi+1)*784) of batch b with zero
cross-core communication; the host concatenates the 8 slices.

Device dataflow is feature-major (features on SBUF partitions, tokens on the
free dim): every linear layer is a natural PE matmul, attention scores are
computed transposed (S^T[j, q]), the rel-pos bias is applied multiplicatively
(exp(s + b) = exp(s) * exp(b), with exp(bias) gathered host-side), and softmax
denominators come from an all-ones matmul whose output is already broadcast
across partitions. LayerNorm affine params and gammas are folded into
downstream weights on the host; K-projection biases drop out exactly via
softmax shift invariance; Q-projection biases enter through the per-key
exp bias column; V-projection biases fold into the output-projection bias
because softmax rows sum to one.
"""

import numpy as np

B, L, C, HEADS = 2, 3136, 384, 3
H, H2 = 56, 28
L2 = L // 4
HD = C // HEADS
EPS = 1e-5
NCORES = 8
QPC = L // 4          # 784 query rows per core
CT = C // 128         # 3 feature tiles
NCH, CHW = 7, 448     # full-L chunking for LN/mlp passes
NQC, QC = 2, 392      # per-core query chunking
TOK2, TT2 = 7, 112    # low-res token tiling (784 = 7*112)
JTS = [(i * 128, 128) for i in range(24)] + [(3072, 64)]   # key tiles
KCH = [(i * 512, 512) for i in range(6)] + [(3072, 64)]    # K/V chunks
LPAD = 3200           # 25*128, padded kb row

_COMPILED = None


def _resize_weight_mat(n_in, n_out):
    # jax.image.resize 'linear' half-pixel: triangle kernel, normalized
    scale = n_out / n_in
    sample_f = (np.arange(n_out) + 0.5) / scale - 0.5
    w = 1.0 - np.abs(sample_f[:, None] - np.arange(n_in)[None, :])
    w = np.clip(w, 0.0, 1.0)
    w = w / w.sum(axis=1, keepdims=True)
    return w.astype(np.float32)


def _host_prep(inp):
    f32 = np.float32
    g = {}
    scale = f32(HD ** -0.5)
    n1w, n1b = inp["n1_w"].astype(f32), inp["n1_b"].astype(f32)
    n2w, n2b = inp["n2_w"].astype(f32), inp["n2_b"].astype(f32)

    def fold_in(w, b, lnw, lnb):
        return (w * lnw[None, :]).astype(f32), (b + w @ lnb).astype(f32)

    wqh, bqh = fold_in(inp["wqh_w"], inp["wqh_b"], n2w, n2b)
    wkh, _ = fold_in(inp["wkh_w"], inp["wkh_b"], n1w, n1b)
    wvh, bvh = fold_in(inp["wvh_w"], inp["wvh_b"], n1w, n1b)
    wql, bql = fold_in(inp["wql_w"], inp["wql_b"], n1w, n1b)
    wkl = inp["wkl_w"].astype(f32)
    wvl, bvl = inp["wvl_w"].astype(f32), inp["wvl_b"].astype(f32)

    g["wqhT"], g["bqh"] = (wqh.T * scale).copy(), bqh * scale
    g["wqlT"], g["bql"] = (wql.T * scale).copy(), bql * scale
    g["wkhT"], g["wklT"] = wkh.T.copy(), wkl.T.copy()
    g["wvhT"], g["wvlT"] = wvh.T.copy(), wvl.T.copy()

    pl1L, pl1R = inp["pl1_w"][:, :C], inp["pl1_w"][:, C:]
    pl1Lw, _ = fold_in(pl1L, np.zeros(C, f32), n2w, n2b)
    pl1Rw, _ = fold_in(pl1R, np.zeros(C, f32), n1w, n1b)
    g["pl1LT"], g["pl1RT"] = pl1Lw.T.copy(), pl1Rw.T.copy()
    g["pl1b"] = (inp["pl1_b"] + pl1L @ n2b + pl1R @ n1b).astype(f32)
    g["pl2T"], g["pl2b"] = inp["pl2_w"].T.copy(), inp["pl2_b"].astype(f32)

    gh, gl = f32(inp["gamma_h"][0]), f32(inp["gamma_l"][0])
    g["fohT"] = (inp["foh_w"].T * gh).astype(f32)
    g["fohb"] = ((inp["foh_b"] + inp["foh_w"] @ bvh) * gh).astype(f32)
    g["folT"] = (inp["fol_w"].T * gl).astype(f32)
    g["folb"] = ((inp["fol_b"] + inp["fol_w"] @ bvl) * gl).astype(f32)

    g["g1LT"] = inp["g1_w"][:, :C].T.copy().astype(f32)
    g["g1RT"] = inp["g1_w"][:, C:].T.copy().astype(f32)
    g["g1b"] = inp["g1_b"].astype(f32)
    g["g2T"] = inp["g2_w"].T.copy().astype(f32)   # [384, 1]
    g["g2b"] = inp["g2_b"].astype(f32)            # [1]

    ffL, ffR = inp["ff_w"][:, :C], inp["ff_w"][:, C:]
    g["ffLT"] = ffL.T.copy().astype(f32)
    g["ffPT"] = (ffL + ffR).T.copy().astype(f32)
    g["ffb"] = inp["ff_b"].astype(f32)

    g["projT"] = inp["proj_w"].T.copy().astype(f32)
    g["projb"] = inp["proj_b"].astype(f32)
    g["penw"], g["penb"] = inp["pen_w"].astype(f32), inp["pen_b"].astype(f32)

    wr = _resize_weight_mat(H2, H)
    g["WupT"] = np.kron(wr, wr).T.copy().astype(f32)  # [784, 3136]

    import ml_dtypes
    expt = np.exp(inp["rpb_table"].astype(f32))       # [12321, 3]
    rel = np.asarray(inp["rel_index"])                # [L, L] int32 (rel[i, j])
    g["expB"] = np.ascontiguousarray(
        expt[rel.T].transpose(2, 0, 1)).astype(ml_dtypes.bfloat16)
    return g


def _build():
    import contextlib
    import concourse.bass as bass  # noqa: F401
    import concourse.tile as tile
    from concourse import bacc, mybir

    f32, bf16, f32r = mybir.dt.float32, mybir.dt.bfloat16, mybir.dt.float32r
    AF = mybir.ActivationFunctionType
    OP = mybir.AluOpType

    nc = bacc.Bacc("TRN2", target_bir_lowering=False, debug=False,
                   num_devices=NCORES)

    def din(name, shape, dtype=f32):
        return nc.dram_tensor(name, shape, dtype, kind="ExternalInput").ap()

    p1T = din("p1T", [C, L], bf16)
    p1T_own = din("p1T_own", [C, QPC], bf16)
    p2T = din("p2T", [2 * C, L2], bf16)
    WupT = din("WupT", [L2, L], bf16)
    WupT_own = din("WupT_own", [L2, QPC], bf16)
    expB = din("expB", [HEADS, L, QPC], bf16)
    w_projT = din("w_projT", [2 * C, C], bf16)
    v_projb = din("v_projb", [C], bf16)
    v_penw, v_penb = din("v_penw", [C]), din("v_penb", [C])
    w_qhT, v_bqh = din("w_qhT", [C, C], bf16), din("v_bqh", [C], f32)
    w_qlT, v_bql = din("w_qlT", [C, C], bf16), din("v_bql", [C], f32)
    v_bqh16, v_bql16 = din("v_bqh16", [C], bf16), din("v_bql16", [C], bf16)
    w_khT, w_klT = din("w_khT", [C, C], bf16), din("w_klT", [C, C], bf16)
    w_vhT, w_vlT = din("w_vhT", [C, C], bf16), din("w_vlT", [C, C], bf16)
    w_pl1LT, w_pl1RT = din("w_pl1LT", [C, C], bf16), din("w_pl1RT", [C, C], bf16)
    v_pl1b = din("v_pl1b", [C])
    w_pl2T, v_pl2b = din("w_pl2T", [C, C], bf16), din("v_pl2b", [C])
    w_fohT, v_fohb = din("w_fohT", [C, C], bf16), din("v_fohb", [C])
    w_folT, v_folb = din("w_folT", [C, C], bf16), din("v_folb", [C])
    w_g1LT, w_g1RT = din("w_g1LT", [C, C], bf16), din("w_g1RT", [C, C], bf16)
    v_g1b = din("v_g1b", [C])
    w_g2T, v_g2b = din("w_g2T", [C, 1], bf16), din("v_g2b", [1])
    w_ffLT, w_ffPT = din("w_ffLT", [C, C], bf16), din("w_ffPT", [C, C], bf16)
    v_ffb = din("v_ffb", [C])

    ones_in = din("ones128", [128, 128], bf16)
    outT = nc.dram_tensor("outT", [C, QPC], f32, kind="ExternalOutput").ap()

    # per-core DRAM scratch for streamed intermediates
    p2up_d = nc.dram_tensor("p2up_d", [C, L], bf16).ap()
    pp_d = nc.dram_tensor("pp_d", [C, L], bf16).ap()
    kb_d = nc.dram_tensor("kb_d", [2, HEADS, LPAD], f32).ap()

    def r32(ap):
        return ap

    with tile.TileContext(nc) as tc:
        with tc.tile_pool(name="const", bufs=1) as const:
            def load_w3(pool, dram, tag, rows=C):
                ts = []
                for k in range(rows // 128):
                    t = pool.tile([128, dram.shape[1]], dram.dtype, tag=f"{tag}_{k}", name=f"{tag}_{k}")
                    nc.sync.dma_start(out=t, in_=dram[k * 128:(k + 1) * 128, :])
                    ts.append(t)
                return ts

            def load_b3(pool, dram, tag, dtype=f32):
                ts = []
                v = dram.rearrange("(a b) -> a b", b=1).bitcast(dtype)
                for k in range(CT):
                    t = pool.tile([128, 1], dtype, tag=f"{tag}_{k}", name=f"{tag}_{k}")
                    nc.sync.dma_start(out=t, in_=v[k * 128:(k + 1) * 128, :])
                    ts.append(t)
                return ts

            ones_f = const.tile([128, 128], bf16, tag="ones_f", name="ones_f")
            nc.sync.dma_start(out=ones_f, in_=ones_in)
            ones_b = const.tile([128, 128], bf16, tag="ones_b", name="ones_b")
            nc.vector.memset(ones_b, 1.0)
            eps_t = const.tile([128, 1], f32, tag="eps_t", name="eps_t")
            nc.vector.memset(eps_t, EPS)

            def ln_feature_major(pool, ppool, n_chunks, chw, src_fn, dst_fn):
                """Feature-major LayerNorm core ((x-m)*r over 384 partitions).
                Stats computed in partition-broadcast form via all-ones
                matmuls."""
                for ch in range(n_chunks):
                    raw = [pool.tile([128, chw], bf16, tag=f"lnraw{cb}", name=f"lnraw{cb}")
                           for cb in range(CT)]
                    for cb in range(CT):
                        src_fn(ch, cb, raw[cb])
                    ps_m = ppool.tile([128, chw], f32, tag="ps_m", name="ps_m")
                    for cb in range(CT):
                        nc.tensor.matmul(ps_m, r32(ones_f), r32(raw[cb]),
                                         start=(cb == 0), stop=(cb == CT - 1))
                    ps_s = ppool.tile([128, chw], f32, tag="ps_s", name="ps_s")
                    for cb in range(CT):
                        sq = pool.tile([128, chw], bf16, tag="lnsq", name="lnsq")
                        nc.scalar.activation(out=sq, in_=raw[cb], func=AF.Square)
                        nc.tensor.matmul(ps_s, r32(ones_f), r32(sq),
                                         start=(cb == 0), stop=(cb == CT - 1))
                    m_bc = pool.tile([128, chw], f32, tag="m_bc", name="m_bc")
                    nc.scalar.activation(out=m_bc, in_=ps_m, func=AF.Copy,
                                         scale=1.0 / C)
                    m2 = pool.tile([128, chw], f32, tag="m2", name="m2")
                    nc.vector.tensor_mul(m2, m_bc, m_bc)
                    v_bc = pool.tile([128, chw], f32, tag="v_bc", name="v_bc")
                    nc.vector.scalar_tensor_tensor(
                        out=v_bc, in0=ps_s, scalar=1.0 / C, in1=m2,
                        op0=OP.mult, op1=OP.subtract)
                    sd = pool.tile([128, chw], f32, tag="lnsd", name="lnsd")
                    nc.scalar.activation(out=sd, in_=v_bc, func=AF.Sqrt,
                                         bias=eps_t, scale=1.0)
                    r_bc = pool.tile([128, chw], f32, tag="r_bc", name="r_bc")
                    rscr = pool.tile([128, chw], f32, tag="lnrscr", name="lnrscr")
                    nc.vector.reciprocal_approx_accurate(out=r_bc, in_=sd,
                                                         scratch=rscr)
                    for cb in range(CT):
                        xc = pool.tile([128, chw], f32, tag="ln_xc", name="ln_xc")
                        nc.vector.tensor_sub(xc, raw[cb], m_bc)
                        xn = pool.tile([128, chw], bf16, tag="ln_xn", name="ln_xn")
                        nc.vector.tensor_mul(xn, xc, r_bc)
                        dst_fn(ch, cb, xn)

            with tc.tile_pool(name="apool", bufs=1) as apool:
                qh = [apool.tile([128, QPC], bf16, tag=f"qh{c}", name=f"qh{c}")
                      for c in range(CT)]
                ql = [apool.tile([128, QPC], bf16, tag=f"ql{c}", name=f"ql{c}")
                      for c in range(CT)]
                oh = [apool.tile([128, QPC], bf16, tag=f"oh{c}", name=f"oh{c}")
                      for c in range(CT)]
                ol = [apool.tile([128, QPC], bf16, tag=f"ol{c}", name=f"ol{c}")
                      for c in range(CT)]

                with tc.tile_pool(name="mid", bufs=1) as mid:
                    xnorm = [mid.tile([TT2, C], bf16, tag=f"xnorm{t}", name=f"xnorm{t}")
                             for t in range(TOK2)]
                    penw3 = load_b3(mid, v_penw, "penw")
                    penb3 = load_b3(mid, v_penb, "penb")

                    # Phase 1: x = LN_pen_core(p2 @ projT + b), token-major
                    with tc.tile_pool(name="ph1s", bufs=1) as ph1s, \
                         tc.tile_pool(name="ph1t", bufs=3) as ph1, \
                         tc.tile_pool(name="ph1p", bufs=2, space="PSUM") as ph1p:
                        tproj = load_w3(ph1s, w_projT, "projT", rows=2 * C)
                        projb_row = ph1s.tile([1, C], bf16, tag="projb_row", name="projb_row")
                        nc.sync.dma_start(
                            out=projb_row,
                            in_=v_projb.rearrange("(a b) -> a b", a=1))
                        p2s = load_w3(ph1s, p2T, "p2s", rows=2 * C)
                        for tt in range(TOK2):
                            ps = ph1p.tile([TT2, C], f32, tag="ps_x", name="ps_x")
                            sl = slice(tt * TT2, (tt + 1) * TT2)
                            for k in range(6):
                                nc.tensor.matmul(ps, r32(p2s[k][:, sl]),
                                                 r32(tproj[k]),
                                                 start=(k == 0), stop=False)
                            nc.tensor.matmul(ps, r32(ones_f[0:1, 0:TT2]),
                                             r32(projb_row),
                                             start=False, stop=True)
                            st = ph1.tile([TT2, 6], f32, tag="bnst", name="bnst")
                            nc.vector.bn_stats(out=st, in_=ps)
                            mv = ph1.tile([TT2, 2], f32, tag="bnmv", name="bnmv")
                            nc.vector.bn_aggr(out=mv, in_=st)
                            sd = ph1.tile([TT2, 1], f32, tag="sd", name="sd")
                            nc.scalar.activation(out=sd, in_=mv[:, 1:2],
                                                 func=AF.Sqrt,
                                                 bias=eps_t[0:TT2], scale=1.0)
                            rr = ph1.tile([TT2, 1], f32, tag="rr", name="rr")
                            rscr = ph1.tile([TT2, 1], f32, tag="rscr", name="rscr")
                            nc.vector.reciprocal_approx_accurate(
                                out=rr, in_=sd, scratch=rscr)
                            nmr = ph1.tile([TT2, 1], f32, tag="nmr", name="nmr")
                            nc.vector.scalar_tensor_tensor(
                                out=nmr, in0=mv[:, 0:1], scalar=-1.0, in1=rr,
                                op0=OP.mult, op1=OP.mult)
                            nc.scalar.activation(out=xnorm[tt], in_=ps,
                                                 func=AF.Identity,
                                                 bias=nmr, scale=rr)

                    # Phase 2: p2_up = LN_n1_core(pen(up(xnorm))) -> DRAM
                    with tc.tile_pool(name="ph2", bufs=2) as ph2, \
                         tc.tile_pool(name="ph2w", bufs=2) as ph2w, \
                         tc.tile_pool(name="ph2p", bufs=2, space="PSUM") as ph2p:
                        wup_cache = {}

                        def up_src(ch, cb, dst):
                            csl = slice(ch * CHW, (ch + 1) * CHW)
                            if cb == 0:
                                wup_cache[ch] = []
                                for kt in range(TOK2):
                                    wt = ph2w.tile([TT2, CHW], bf16,
                                                   tag=f"wup{kt}", name=f"wup{kt}")
                                    nc.sync.dma_start(
                                        out=wt,
                                        in_=WupT[kt * TT2:(kt + 1) * TT2, csl])
                                    wup_cache[ch].append(wt)
                            ps = ph2p.tile([128, CHW], f32, tag="ps_up", name="ps_up")
                            for kt in range(TOK2):
                                nc.tensor.matmul(
                                    ps,
                                    r32(xnorm[kt][:, cb * 128:(cb + 1) * 128]),
                                    r32(wup_cache[ch][kt]),
                                    start=(kt == 0), stop=(kt == TOK2 - 1))
                            nc.scalar.activation(out=dst, in_=ps,
                                                 func=AF.Identity,
                                                 bias=penb3[cb],
                                                 scale=penw3[cb])

                        def up_dst(ch, cb, t):
                            csl = slice(ch * CHW, (ch + 1) * CHW)
                            nc.sync.dma_start(
                                out=p2up_d[cb * 128:(cb + 1) * 128, csl],
                                in_=t)

                        ln_feature_major(ph2, ph2p, NCH, CHW, up_src, up_dst)

                    # Phase 3+4: p1_n (SBUF) then pp -> DRAM
                    with tc.tile_pool(name="p1npool", bufs=1) as p1npool:
                        p1n = [p1npool.tile([128, L], bf16, tag=f"p1n{c}", name=f"p1n{c}")
                               for c in range(CT)]
                        with tc.tile_pool(name="ph3", bufs=2) as ph3, \
                             tc.tile_pool(name="ph3p", bufs=2,
                                          space="PSUM") as ph3p:

                            def p1_src(ch, cb, dst):
                                csl = slice(ch * CHW, (ch + 1) * CHW)
                                nc.sync.dma_start(
                                    out=dst,
                                    in_=p1T[cb * 128:(cb + 1) * 128, csl])

                            def p1_dst(ch, cb, t):
                                csl = slice(ch * CHW, (ch + 1) * CHW)
                                nc.vector.tensor_copy(p1n[cb][:, csl], t)

                            ln_feature_major(ph3, ph3p, NCH, CHW, p1_src,
                                             p1_dst)

                        with tc.tile_pool(name="ph4w", bufs=1) as ph4w, \
                             tc.tile_pool(name="ph4", bufs=2) as ph4, \
                             tc.tile_pool(name="ph4p", bufs=3,
                                          space="PSUM") as ph4p:
                            tl1L = load_w3(ph4w, w_pl1LT, "pl1LT")
                            tl1R = load_w3(ph4w, w_pl1RT, "pl1RT")
                            tl2 = load_w3(ph4w, w_pl2T, "pl2T")
                            bl1 = load_b3(ph4w, v_pl1b, "pl1b")
                            bl2 = load_b3(ph4w, v_pl2b, "pl2b")
                            for ch in range(NCH):
                                csl = slice(ch * CHW, (ch + 1) * CHW)
                                up3 = []
                                for cb in range(CT):
                                    t = ph4.tile([128, CHW], bf16,
                                                 tag=f"up3_{cb}", name=f"up3_{cb}")
                                    nc.sync.dma_start(
                                        out=t,
                                        in_=p2up_d[cb * 128:(cb + 1) * 128,
                                                   csl])
                                    up3.append(t)
                                gel = []
                                for cb in range(CT):
                                    ps = ph4p.tile([128, CHW], f32,
                                                   tag="ps_pp1", name="ps_pp1")
                                    for kt in range(CT):
                                        nc.tensor.matmul(
                                            ps,
                                            r32(tl1L[kt][:, cb * 128:(cb + 1) * 128]),
                                            r32(p1n[kt][:, csl]),
                                            start=(kt == 0), stop=False)
                                    for kt in range(CT):
                                        nc.tensor.matmul(
                                            ps,
                                            r32(tl1R[kt][:, cb * 128:(cb + 1) * 128]),
                                            r32(up3[kt]), start=False,
                                            stop=(kt == CT - 1))
                                    gt = ph4.tile([128, CHW], bf16,
                                                  tag=f"gel{cb}", name=f"gel{cb}")
                                    nc.scalar.activation(out=gt, in_=ps,
                                                         func=AF.Gelu,
                                                         bias=bl1[cb],
                                                         scale=1.0)
                                    gel.append(gt)
                                for cb in range(CT):
                                    ps = ph4p.tile([128, CHW], f32,
                                                   tag="ps_pp2", name="ps_pp2")
                                    for kt in range(CT):
                                        nc.tensor.matmul(
                                            ps,
                                            r32(tl2[kt][:, cb * 128:(cb + 1) * 128]),
                                            r32(gel[kt]), start=(kt == 0),
                                            stop=(kt == CT - 1))
                                    ot = ph4.tile([128, CHW], bf16, tag="ppo", name="ppo")
                                    nc.scalar.activation(out=ot, in_=ps,
                                                         func=AF.Identity,
                                                         bias=bl2[cb],
                                                         scale=1.0)
                                    nc.sync.dma_start(
                                        out=pp_d[cb * 128:(cb + 1) * 128, csl],
                                        in_=ot)

                    # Phase 5: own-slice recompute + Q projections
                    with tc.tile_pool(name="ph5s", bufs=1) as ph5s, \
                         tc.tile_pool(name="ph5", bufs=2) as ph5, \
                         tc.tile_pool(name="ph5w", bufs=2) as ph5w, \
                         tc.tile_pool(name="ph5p", bufs=2, space="PSUM") as ph5p:
                        tqh = load_w3(ph5s, w_qhT, "qhT")
                        bqh3 = load_b3(ph5s, v_bqh, "bqh")
                        tql = load_w3(ph5s, w_qlT, "qlT")
                        bql3 = load_b3(ph5s, v_bql, "bql")
                        p1o5 = [ph5s.tile([128, QPC], bf16, tag=f"p1o5{c}", name=f"p1o5{c}")
                                for c in range(CT)]
                        for cb in range(CT):
                            nc.sync.dma_start(
                                out=p1o5[cb],
                                in_=p1T_own[cb * 128:(cb + 1) * 128, :])
                        p2upo = [ph5s.tile([128, QPC], bf16, tag=f"p2upo{c}", name=f"p2upo{c}")
                                 for c in range(CT)]
                        p1no = [ph5s.tile([128, QPC], bf16, tag=f"p1no{c}", name=f"p1no{c}")
                                for c in range(CT)]
                        wupo_cache = {}

                        def upo_src(ch, cb, dst):
                            csl = slice(ch * QC, (ch + 1) * QC)
                            if cb == 0:
                                wupo_cache[ch] = []
                                for kt in range(TOK2):
                                    wt = ph5w.tile([TT2, QC], bf16,
                                                   tag=f"wupo{kt}", name=f"wupo{kt}")
                                    nc.sync.dma_start(
                                        out=wt,
                                        in_=WupT_own[kt * TT2:(kt + 1) * TT2,
                                                     csl])
                                    wupo_cache[ch].append(wt)
                            ps = ph5p.tile([128, QC], f32, tag="ps_upo", name="ps_upo")
                            for kt in range(TOK2):
                                nc.tensor.matmul(
                                    ps,
                                    r32(xnorm[kt][:, cb * 128:(cb + 1) * 128]),
                                    r32(wupo_cache[ch][kt]),
                                    start=(kt == 0), stop=(kt == TOK2 - 1))
                            nc.scalar.activation(out=dst, in_=ps,
                                                 func=AF.Identity,
                                                 bias=penb3[cb],
                                                 scale=penw3[cb])

                        ln_feature_major(
                            ph5, ph5p, NQC, QC, upo_src,
                            lambda ch, cb, t: nc.vector.tensor_copy(
                                p2upo[cb][:, ch * QC:(ch + 1) * QC], t))

                        def p1o_src(ch, cb, dst):
                            nc.vector.tensor_copy(
                                dst, p1o5[cb][:, ch * QC:(ch + 1) * QC])

                        ln_feature_major(
                            ph5, ph5p, NQC, QC, p1o_src,
                            lambda ch, cb, t: nc.vector.tensor_copy(
                                p1no[cb][:, ch * QC:(ch + 1) * QC], t))

                        for (dst, src, tw, tb) in ((qh, p1no, tqh, bqh3),
                                                   (ql, p2upo, tql, bql3)):
                            for ch in range(NQC):
                                csl = slice(ch * QC, (ch + 1) * QC)
                                for cb in range(CT):
                                    ps = ph5p.tile([128, QC], f32, tag="ps_q", name="ps_q")
                                    for kt in range(CT):
                                        nc.tensor.matmul(
                                            ps,
                                            r32(tw[kt][:, cb * 128:(cb + 1) * 128]),
                                            r32(src[kt][:, csl]),
                                            start=(kt == 0),
                                            stop=(kt == CT - 1))
                                    nc.scalar.activation(
                                        out=dst[cb][:, csl], in_=ps,
                                        func=AF.Identity, bias=tb[cb],
                                        scale=1.0)

                # Phase 6: K (feature-major) and V (token-major bf16)
                with tc.tile_pool(name="kvpool", bufs=1) as kvpool:
                    kh = [kvpool.tile([128, L], bf16, tag=f"kh{c}", name=f"kh{c}")
                          for c in range(CT)]
                    kl = [kvpool.tile([128, L], bf16, tag=f"kl{c}", name=f"kl{c}")
                          for c in range(CT)]
                    vh = [kvpool.tile([jn, C], bf16, tag=f"vh{i}", name=f"vh{i}")
                          for i, (_, jn) in enumerate(JTS)]
                    vl = [kvpool.tile([jn, C], bf16, tag=f"vl{i}", name=f"vl{i}")
                          for i, (_, jn) in enumerate(JTS)]

                    with tc.tile_pool(name="ph6w", bufs=1) as ph6w, \
                         tc.tile_pool(name="ph6", bufs=2) as ph6, \
                         tc.tile_pool(name="ph6p", bufs=2, space="PSUM") as ph6p:
                        tkh = load_w3(ph6w, w_khT, "khT")
                        tkl = load_w3(ph6w, w_klT, "klT")
                        tvh = load_w3(ph6w, w_vhT, "vhT")
                        tvl = load_w3(ph6w, w_vlT, "vlT")
                        for (kk, vv, srcd, twk, twv) in (
                                (kh, vh, p2up_d, tkh, tvh),
                                (kl, vl, pp_d, tkl, tvl)):
                            for ci, (c0, cw) in enumerate(KCH):
                                s3 = []
                                for cb in range(CT):
                                    t = ph6.tile([128, cw], bf16,
                                                 tag=f"kv_src{cb}", name=f"kv_src{cb}")
                                    nc.sync.dma_start(
                                        out=t,
                                        in_=srcd[cb * 128:(cb + 1) * 128,
                                                 c0:c0 + cw])
                                    s3.append(t)
                                for cb in range(CT):
                                    ps = ph6p.tile([128, cw], f32, tag="ps_k", name="ps_k")
                                    for kt in range(CT):
                                        nc.tensor.matmul(
                                            ps,
                                            r32(twk[kt][:, cb * 128:(cb + 1) * 128]),
                                            r32(s3[kt]), start=(kt == 0),
                                            stop=(kt == CT - 1))
                                    nc.scalar.copy(out=kk[cb][:, c0:c0 + cw],
                                                   in_=ps)
                                for sub in range(max(1, cw // 128)):
                                    off = sub * 128
                                    jn = min(128, cw - off)
                                    vi = (c0 + off) // 128
                                    ps = ph6p.tile([128, C], f32, tag="ps_v", name="ps_v")
                                    for kt in range(CT):
                                        nc.tensor.matmul(
                                            ps[:jn],
                                            r32(s3[kt][:, off:off + jn]),
                                            r32(twv[kt]), start=(kt == 0),
                                            stop=(kt == CT - 1))
                                    nc.vector.tensor_copy(vv[vi], ps[:jn])

                    # Phase 7: attention
                    with tc.tile_pool(name="atw", bufs=1) as atw:
                        tfoh = load_w3(atw, w_fohT, "fohT")
                        bfoh = load_b3(atw, v_fohb, "fohb")
                        tfol = load_w3(atw, w_folT, "folT")
                        bfol = load_b3(atw, v_folb, "folb")
                        bqh3b = load_b3(atw, v_bqh16, "bqhB", dtype=bf16)
                        bql3b = load_b3(atw, v_bql16, "bqlB", dtype=bf16)

                        # kb[j] = K_h[:, j] . bq_h  -> [128, 25] column layout
                        kbcol = {}
                        with tc.tile_pool(name="kbp", bufs=2) as kbp, \
                             tc.tile_pool(name="kbps", bufs=2,
                                          space="PSUM") as kbps:
                            for a, (kk, bq) in enumerate(((kh, bqh3b),
                                                          (kl, bql3b))):
                                for h in range(HEADS):
                                    row = kbp.tile([1, LPAD], f32,
                                                   tag="kbrow", name="kbrow")
                                    nc.vector.memset(row, 0.0)
                                    for ch in range(NCH):
                                        csl = slice(ch * CHW, (ch + 1) * CHW)
                                        ps = kbps.tile([1, CHW], f32,
                                                       tag="ps_kb", name="ps_kb")
                                        nc.tensor.matmul(
                                            ps, r32(bq[h]), r32(kk[h][:, csl]),
                                            start=True, stop=True)
                                        nc.vector.tensor_copy(row[:, csl], ps)
                                    nc.sync.dma_start(
                                        out=kb_d[a, h, :].rearrange(
                                            "(o n) -> o n", o=1),
                                        in_=row)
                                    col = atw.tile([128, len(JTS)], f32,
                                                   tag=f"kbcol{a}{h}", name=f"kbcol{a}{h}")
                                    nc.sync.dma_start(
                                        out=col,
                                        in_=kb_d[a, h, :].rearrange(
                                            "(t p) -> p t", p=128))
                                    kbcol[(a, h)] = col

                        with tc.tile_pool(name="at", bufs=3) as at, \
                             tc.tile_pool(name="atb", bufs=3) as atb, \
                             tc.tile_pool(name="ato", bufs=1) as ato, \
                             tc.tile_pool(name="atps", bufs=2, space="PSUM") as atps, \
                             tc.tile_pool(name="atpo", bufs=2, space="PSUM") as atpo, \
                             tc.tile_pool(name="atpd", bufs=2, space="PSUM") as atpd, \
                             tc.tile_pool(name="atpp", bufs=2, space="PSUM") as atpp:
                            for qc in range(NQC):
                                qsl = slice(qc * QC, (qc + 1) * QC)
                                onorm = {}
                                for h in range(HEADS):
                                    ps_o = [atpo.tile([128, QC], f32, tag="ps_o", name="ps_o")
                                            for _ in range(2)]
                                    ps_d = [atpd.tile([128, QC], f32, tag="ps_d", name="ps_d")
                                            for _ in range(2)]
                                    for i, (j0, jn) in enumerate(JTS):
                                        eb = atb.tile([jn, QC], bf16, tag="eb", name="eb")
                                        nc.sync.dma_start(
                                            out=eb, in_=expB[h, j0:j0 + jn, qsl])
                                        for a, (kk, qq, vv) in enumerate(
                                                ((kh, qh, vh), (kl, ql, vl))):
                                            ps_s = atps.tile([jn, QC], f32,
                                                             tag="ps_s", name="ps_s")
                                            nc.tensor.matmul(
                                                ps_s, r32(kk[h][:, j0:j0 + jn]),
                                                r32(qq[h][:, qsl]),
                                                start=True, stop=True)
                                            ee = at.tile([jn, QC], bf16, tag="ee", name="ee")
                                            nc.scalar.activation(
                                                out=ee, in_=ps_s, func=AF.Exp,
                                                bias=kbcol[(a, h)][:jn, i:i + 1],
                                                scale=1.0)
                                            aa = at.tile([jn, QC], bf16, tag="aa", name="aa")
                                            nc.vector.tensor_mul(aa, ee, eb)
                                            nc.tensor.matmul(
                                                ps_o[a],
                                                vv[i][:, h * 128:(h + 1) * 128],
                                                aa, start=(i == 0),
                                                stop=(i == len(JTS) - 1))
                                            nc.tensor.matmul(
                                                ps_d[a], ones_b[:jn], aa,
                                                start=(i == 0),
                                                stop=(i == len(JTS) - 1))
                                    for a in range(2):
                                        rden = at.tile([128, QC], f32, tag="rden", name="rden")
                                        nc.vector.reciprocal_approx_fast(
                                            out=rden, in_=ps_d[a])
                                        on = ato.tile([128, QC], bf16,
                                                      tag=f"on{a}{h}", name=f"on{a}{h}")
                                        nc.vector.tensor_mul(on, ps_o[a], rden)
                                        onorm[(a, h)] = on
                                for a, (dst, tw, tb) in enumerate(
                                        ((oh, tfoh, bfoh), (ol, tfol, bfol))):
                                    for cb in range(CT):
                                        ps = atpp.tile([128, QC], f32,
                                                       tag="ps_fo", name="ps_fo")
                                        for h in range(HEADS):
                                            nc.tensor.matmul(
                                                ps,
                                                r32(tw[h][:, cb * 128:(cb + 1) * 128]),
                                                r32(onorm[(a, h)]),
                                                start=(h == 0),
                                                stop=(h == HEADS - 1))
                                        nc.scalar.activation(
                                            out=dst[cb][:, qsl], in_=ps,
                                            func=AF.Identity, bias=tb[cb],
                                            scale=1.0)

                # Phase 8: gate, mix, ff
                with tc.tile_pool(name="ph8w", bufs=1) as ph8w, \
                     tc.tile_pool(name="ph8", bufs=2) as ph8, \
                     tc.tile_pool(name="ph8p", bufs=2, space="PSUM") as ph8p:
                    tg1L = load_w3(ph8w, w_g1LT, "g1LT")
                    tg1R = load_w3(ph8w, w_g1RT, "g1RT")
                    bg1 = load_b3(ph8w, v_g1b, "g1b")
                    tg2 = load_w3(ph8w, w_g2T, "g2T")
                    g2b_t = ph8w.tile([1, 1], f32, tag="g2b_t", name="g2b_t")
                    nc.sync.dma_start(
                        out=g2b_t, in_=v_g2b.rearrange("(a b) -> a b", a=1))
                    tffL = load_w3(ph8w, w_ffLT, "ffLT")
                    tffP = load_w3(ph8w, w_ffPT, "ffPT")
                    bff = load_b3(ph8w, v_ffb, "ffb")
                    p1o = [ph8w.tile([128, QPC], bf16, tag=f"p1o{c}", name=f"p1o{c}")
                           for c in range(CT)]
                    for cb in range(CT):
                        nc.sync.dma_start(
                            out=p1o[cb],
                            in_=p1T_own[cb * 128:(cb + 1) * 128, :])
                    for qc in range(NQC):
                        qsl = slice(qc * QC, (qc + 1) * QC)
                        gel = []
                        for cb in range(CT):
                            ps = ph8p.tile([128, QC], f32, tag="ps_g1", name="ps_g1")
                            for kt in range(CT):
                                nc.tensor.matmul(
                                    ps,
                                    r32(tg1L[kt][:, cb * 128:(cb + 1) * 128]),
                                    r32(oh[kt][:, qsl]),
                                    start=(kt == 0), stop=False)
                            for kt in range(CT):
                                nc.tensor.matmul(
                                    ps,
                                    r32(tg1R[kt][:, cb * 128:(cb + 1) * 128]),
                                    r32(ol[kt][:, qsl]), start=False,
                                    stop=(kt == CT - 1))
                            gt = ph8.tile([128, QC], bf16, tag=f"ggel{cb}", name=f"ggel{cb}")
                            nc.scalar.activation(out=gt, in_=ps, func=AF.Gelu,
                                                 bias=bg1[cb], scale=1.0)
                            gel.append(gt)
                        ps_z = ph8p.tile([1, QC], f32, tag="ps_z", name="ps_z")
                        for kt in range(CT):
                            nc.tensor.matmul(ps_z, r32(tg2[kt]), r32(gel[kt]),
                                             start=(kt == 0),
                                             stop=(kt == CT - 1))
                        gate = ph8.tile([1, QC], bf16, tag="gate", name="gate")
                        nc.scalar.activation(out=gate, in_=ps_z,
                                             func=AF.Sigmoid,
                                             bias=g2b_t, scale=1.0)
                        ps_gb = ph8p.tile([128, QC], f32, tag="ps_gb", name="ps_gb")
                        nc.tensor.matmul(ps_gb, r32(ones_f[0:1, :]), r32(gate),
                                         start=True, stop=True)
                        mix = []
                        for cb in range(CT):
                            dd = ph8.tile([128, QC], f32, tag="dd", name="dd")
                            nc.vector.tensor_sub(dd, oh[cb][:, qsl],
                                                 ol[cb][:, qsl])
                            d2 = ph8.tile([128, QC], f32, tag="d2", name="d2")
                            nc.vector.tensor_mul(d2, dd, ps_gb)
                            mx = ph8.tile([128, QC], bf16, tag=f"mix{cb}", name=f"mix{cb}")
                            nc.vector.tensor_add(mx, d2, ol[cb][:, qsl])
                            mix.append(mx)
                        for cb in range(CT):
                            ps = ph8p.tile([128, QC], f32, tag="ps_ff", name="ps_ff")
                            for kt in range(CT):
                                nc.tensor.matmul(
                                    ps,
                                    r32(tffL[kt][:, cb * 128:(cb + 1) * 128]),
                                    r32(mix[kt]), start=(kt == 0), stop=False)
                            for kt in range(CT):
                                nc.tensor.matmul(
                                    ps,
                                    r32(tffP[kt][:, cb * 128:(cb + 1) * 128]),
                                    r32(p1o[kt][:, qsl]), start=False,
                                    stop=(kt == CT - 1))
                            res = ph8.tile([128, QC], f32, tag="res", name="res")
                            nc.scalar.activation(out=res, in_=ps,
                                                 func=AF.Identity,
                                                 bias=bff[cb], scale=1.0)
                            nc.sync.dma_start(
                                out=outT[cb * 128:(cb + 1) * 128, qsl],
                                in_=res)

    nc.compile()
    return nc


def _prepare(inputs):
    """Host prep + input sharding. Returns (nc, in_maps)."""
    global _COMPILED
    inp = {k: np.asarray(v) for k, v in inputs.items()}
    g = _host_prep(inp)

    if _COMPILED is None:
        _COMPILED = _build()
    nc = _COMPILED

    import ml_dtypes
    bf16 = ml_dtypes.bfloat16
    p1 = inp["p1"].astype(np.float32)
    p2 = inp["p2"].astype(np.float32)
    shared = {
        "ones128": np.ones((128, 128), bf16),
        "WupT": g["WupT"].astype(bf16),
        "w_projT": g["projT"].astype(bf16),
        "v_projb": g["projb"].astype(bf16),
        "v_penw": g["penw"], "v_penb": g["penb"],
        "w_qhT": g["wqhT"].astype(bf16), "v_bqh": g["bqh"],
        "w_qlT": g["wqlT"].astype(bf16), "v_bql": g["bql"],
        "v_bqh16": g["bqh"].astype(bf16), "v_bql16": g["bql"].astype(bf16),
        "w_khT": g["wkhT"].astype(bf16), "w_klT": g["wklT"].astype(bf16),
        "w_vhT": g["wvhT"].astype(bf16), "w_vlT": g["wvlT"].astype(bf16),
        "w_pl1LT": g["pl1LT"].astype(bf16), "w_pl1RT": g["pl1RT"].astype(bf16),
        "v_pl1b": g["pl1b"],
        "w_pl2T": g["pl2T"].astype(bf16), "v_pl2b": g["pl2b"],
        "w_fohT": g["fohT"].astype(bf16), "v_fohb": g["fohb"],
        "w_folT": g["folT"].astype(bf16), "v_folb": g["folb"],
        "w_g1LT": g["g1LT"].astype(bf16), "w_g1RT": g["g1RT"].astype(bf16),
        "v_g1b": g["g1b"],
        "w_g2T": g["g2T"].astype(bf16), "v_g2b": g["g2b"],
        "w_ffLT": g["ffLT"].astype(bf16), "w_ffPT": g["ffPT"].astype(bf16),
        "v_ffb": g["ffb"],
    }
    shared = {k: np.ascontiguousarray(v) for k, v in shared.items()}

    in_maps = []
    for core in range(NCORES):
        b, qi = divmod(core, 4)
        q0 = qi * QPC
        m = dict(shared)
        m["p1T"] = np.ascontiguousarray(p1[b].T.astype(bf16))
        m["p1T_own"] = np.ascontiguousarray(p1[b, q0:q0 + QPC, :].T.astype(bf16))
        m["p2T"] = np.ascontiguousarray(p2[b].T.astype(bf16))
        m["WupT_own"] = np.ascontiguousarray(g["WupT"][:, q0:q0 + QPC].astype(bf16))
        m["expB"] = np.ascontiguousarray(g["expB"][:, :, q0:q0 + QPC])
        in_maps.append(m)

    return nc, in_maps


def _run(nc, in_maps):
    from concourse.bass_utils import run_bass_kernel_spmd
    res = run_bass_kernel_spmd(nc, in_maps, core_ids=list(range(NCORES)))
    out = np.zeros((B, L, C), np.float32)
    for core in range(NCORES):
        b, qi = divmod(core, 4)
        q0 = qi * QPC
        out[b, q0:q0 + QPC, :] = res.results[core]["outT"].T
    return out


def kernel(**inputs):
    nc, in_maps = _prepare(inputs)
    return _run(nc, in_maps)



# revision 6
# speedup vs baseline: 1.2216x; 1.2216x over previous
"""Trainium2 Bass kernel for nn_CrossfusionBidirectional — v2 (restructured).

Sharding: 8 cores = (batch b in {0,1}) x (query-row quarter qi in {0..3}).

v2 changes vs baseline:
- all matmuls bf16 (fp32 PSUM accumulate)
- bilinear 2x upsample computed as 8 strided vector ops per feature tile
  (weights {0.75,0.25} * 4 folded to integer stencil 3x+y with a /16 that
  LayerNorm absorbs via a pre-scaled pen weight), replacing the dense
  [784x3136] upsample matmul
- one fused streaming chunk loop (512 tokens) produces p1n / p2up-LN / pp /
  K / V / kb entirely in SBUF: no DRAM roundtrips between phases
- own-quarter path recomputed from small per-core inputs (p1T_own, Wup_ownT)
- attention software-pipelined: S-matmuls of key-tile i are emitted before
  the AV/denominator matmuls of tile i-1, hiding the exp+mul latency
- expB pre-tiled host-side into contiguous [128, 392] blocks (single
  descriptor DMA per tile)
"""

import numpy as np

B, L, C, HEADS = 2, 3136, 384, 3
H, H2 = 56, 28
L2 = L // 4
HD = C // HEADS
EPS = 1e-5
NCORES = 8
QPC = L // 4          # 784 query rows per core
CT = C // 128         # 3 feature tiles
NQC, QC = 2, 392      # per-core query chunking
TOK2, TT2 = 7, 112    # low-res token tiling (784 = 7*112)
JTS = [(i * 128, 128) for i in range(24)] + [(3072, 64)]   # key tiles
NJT = len(JTS)
KCH = [(i * 512, 512) for i in range(6)] + [(3072, 64)]    # stream chunks
LPAD = 3200           # 25*128, padded kb row

_COMPILED = None


def _resize_weight_mat(n_in, n_out):
    scale = n_out / n_in
    sample_f = (np.arange(n_out) + 0.5) / scale - 0.5
    w = 1.0 - np.abs(sample_f[:, None] - np.arange(n_in)[None, :])
    w = np.clip(w, 0.0, 1.0)
    w = w / w.sum(axis=1, keepdims=True)
    return w.astype(np.float32)


def _host_prep(inp):
    f32 = np.float32
    g = {}
    scale = f32(HD ** -0.5)
    n1w, n1b = inp["n1_w"].astype(f32), inp["n1_b"].astype(f32)
    n2w, n2b = inp["n2_w"].astype(f32), inp["n2_b"].astype(f32)

    def fold_in(w, b, lnw, lnb):
        return (w * lnw[None, :]).astype(f32), (b + w @ lnb).astype(f32)

    wqh, bqh = fold_in(inp["wqh_w"], inp["wqh_b"], n2w, n2b)
    wkh, _ = fold_in(inp["wkh_w"], inp["wkh_b"], n1w, n1b)
    wvh, bvh = fold_in(inp["wvh_w"], inp["wvh_b"], n1w, n1b)
    wql, bql = fold_in(inp["wql_w"], inp["wql_b"], n1w, n1b)
    wkl = inp["wkl_w"].astype(f32)
    wvl, bvl = inp["wvl_w"].astype(f32), inp["wvl_b"].astype(f32)

    g["wqhT"], g["bqh"] = (wqh.T * scale).copy(), bqh * scale
    g["wqlT"], g["bql"] = (wql.T * scale).copy(), bql * scale
    g["wkhT"], g["wklT"] = wkh.T.copy(), wkl.T.copy()
    g["wvhT"], g["wvlT"] = wvh.T.copy(), wvl.T.copy()

    pl1L, pl1R = inp["pl1_w"][:, :C], inp["pl1_w"][:, C:]
    pl1Lw, _ = fold_in(pl1L, np.zeros(C, f32), n2w, n2b)
    pl1Rw, _ = fold_in(pl1R, np.zeros(C, f32), n1w, n1b)
    g["pl1LT"], g["pl1RT"] = pl1Lw.T.copy(), pl1Rw.T.copy()
    g["pl1b"] = (inp["pl1_b"] + pl1L @ n2b + pl1R @ n1b).astype(f32)
    g["pl2T"], g["pl2b"] = inp["pl2_w"].T.copy(), inp["pl2_b"].astype(f32)

    gh, gl = f32(inp["gamma_h"][0]), f32(inp["gamma_l"][0])
    g["fohT"] = (inp["foh_w"].T * gh).astype(f32)
    g["fohb"] = ((inp["foh_b"] + inp["foh_w"] @ bvh) * gh).astype(f32)
    g["folT"] = (inp["fol_w"].T * gl).astype(f32)
    g["folb"] = ((inp["fol_b"] + inp["fol_w"] @ bvl) * gl).astype(f32)

    g["g1LT"] = inp["g1_w"][:, :C].T.copy().astype(f32)
    g["g1RT"] = inp["g1_w"][:, C:].T.copy().astype(f32)
    g["g1b"] = inp["g1_b"].astype(f32)
    g["g2T"] = inp["g2_w"].T.copy().astype(f32)   # [384, 1]
    g["g2b"] = inp["g2_b"].astype(f32)            # [1]

    ffL, ffR = inp["ff_w"][:, :C], inp["ff_w"][:, C:]
    g["ffLT"] = ffL.T.copy().astype(f32)
    g["ffPT"] = (ffL + ffR).T.copy().astype(f32)
    g["ffb"] = inp["ff_b"].astype(f32)

    g["projT"] = inp["proj_w"].T.copy().astype(f32)
    g["projb"] = inp["proj_b"].astype(f32)
    g["penw"], g["penb"] = inp["pen_w"].astype(f32), inp["pen_b"].astype(f32)

    wr = _resize_weight_mat(H2, H)
    g["WupT"] = np.kron(wr, wr).T.copy().astype(np.float32)  # [784, 3136]

    import ml_dtypes
    expt = np.exp(inp["rpb_table"].astype(f32))       # [12321, 3]
    rel = np.asarray(inp["rel_index"])                # [L, L] int32 (rel[i, j])
    # expB[h, j, i] = exp(table[rel[i, j], h])  (keys j, queries i)
    g["expB"] = np.ascontiguousarray(
        expt[rel.T].transpose(2, 0, 1)).astype(ml_dtypes.bfloat16)
    return g


def _build():
    import concourse.bass as bass  # noqa: F401
    import concourse.tile as tile
    from concourse import bacc, mybir

    f32, bf16 = mybir.dt.float32, mybir.dt.bfloat16
    AF = mybir.ActivationFunctionType
    OP = mybir.AluOpType

    nc = bacc.Bacc("TRN2", target_bir_lowering=False, debug=False,
                   num_devices=NCORES)

    def din(name, shape, dtype=f32):
        return nc.dram_tensor(name, shape, dtype, kind="ExternalInput").ap()

    p1Tt = din("p1Tt", [128, CT, L], bf16)
    p1T_own = din("p1T_own", [C, QPC], bf16)
    ph1cat = din("ph1cat", [2 * C, C + L2], bf16)   # [projT | p2T]
    wupcat = din("wupcat", [TT2, TOK2 * QPC], bf16)
    expBt = din("expBt", [NQC, HEADS, NJT, 128, QC], bf16)
    eye_in = din("eye128", [128, 128], bf16)
    v_projb = din("v_projb", [C], bf16)
    # packed C x C weights: [khT klT vhT vlT pl1LT pl1RT pl2T qhT qlT]
    wcat_sw = din("wcat_sw", [C, 9 * C], bf16)
    # packed: [fohT folT g1LT g1RT ffLT ffPT g2T]
    wcat_at = din("wcat_at", [C, 6 * C + 1], bf16)
    # packed f32 biases, column j*3+k = bias_j[k*128:(k+1)*128]:
    # [penw penb penw16 pl1b pl2b bqh bql fohb folb g1b ffb]
    bcat = din("bcat", [128, 33], f32)
    bcat16 = din("bcat16", [128, 6], bf16)          # [bqh16 bql16]
    v_g2b = din("v_g2b", [1])

    outT = nc.dram_tensor("outT", [C, QPC], f32, kind="ExternalOutput").ap()
    kb_d = nc.dram_tensor("kb_d", [2, HEADS, LPAD], f32).ap()

    with tile.TileContext(nc) as tc:
        with tc.tile_pool(name="const", bufs=1) as const:
            def load_cat(pool, dram, tag, rows=C):
                ts = []
                for k in range(rows // 128):
                    t = pool.tile([128, dram.shape[1]], dram.dtype,
                                  tag=f"{tag}_{k}", name=f"{tag}_{k}")
                    nc.sync.dma_start(out=t, in_=dram[k * 128:(k + 1) * 128, :])
                    ts.append(t)
                return ts

            def wslice(cat_tiles, j):
                return [t[:, j * C:(j + 1) * C] for t in cat_tiles]

            def bslice(bc_tiles, j):
                return [bc_tiles[:, 3 * j + k:3 * j + k + 1] for k in range(CT)]

            ones_f = const.tile([128, 128], bf16, tag="ones_f", name="ones_f")
            nc.vector.memset(ones_f, 1.0)
            eye_t = const.tile([128, 128], bf16, tag="eye_t", name="eye_t")
            nc.sync.dma_start(out=eye_t, in_=eye_in)
            eps_t = const.tile([128, 1], f32, tag="eps_t", name="eps_t")
            nc.vector.memset(eps_t, EPS)
            bct = const.tile([128, 33], f32, tag="bct", name="bct")
            nc.sync.dma_start(out=bct, in_=bcat)
            bct16 = const.tile([128, 6], bf16, tag="bct16", name="bct16")
            nc.sync.dma_start(out=bct16, in_=bcat16)
            penw3 = bslice(bct, 0)
            penb3 = bslice(bct, 1)
            penw163 = bslice(bct, 2)

            def ln_fm_chunk(pool, ppool, srcs, cw, dst_fn, tagp):
                """Feature-major LN on one chunk: srcs = 3 x AP [128, cw]
                (bf16). dst_fn(cb) -> AP written with the normalized bf16
                result. Stats via ones-matmuls (partition-broadcast)."""
                ps_m = ppool.tile([128, cw], f32, tag=f"{tagp}_psm", name=f"{tagp}_psm")
                for cb in range(CT):
                    nc.tensor.matmul(ps_m, ones_f, srcs[cb],
                                     start=(cb == 0), stop=(cb == CT - 1))
                ps_s = ppool.tile([128, cw], f32, tag=f"{tagp}_pss", name=f"{tagp}_pss")
                for cb in range(CT):
                    sq = pool.tile([128, cw], bf16, tag=f"{tagp}_sq", name=f"{tagp}_sq")
                    nc.scalar.activation(out=sq, in_=srcs[cb], func=AF.Square)
                    nc.tensor.matmul(ps_s, ones_f, sq,
                                     start=(cb == 0), stop=(cb == CT - 1))
                m_bc = pool.tile([128, cw], f32, tag=f"{tagp}_mbc", name=f"{tagp}_mbc")
                nc.scalar.activation(out=m_bc, in_=ps_m, func=AF.Copy,
                                     scale=1.0 / C)
                m2 = pool.tile([128, cw], bf16, tag=f"{tagp}_m2", name=f"{tagp}_m2")
                nc.vector.tensor_mul(m2, m_bc, m_bc)
                v_bc = pool.tile([128, cw], f32, tag=f"{tagp}_vbc", name=f"{tagp}_vbc")
                nc.vector.scalar_tensor_tensor(
                    out=v_bc, in0=ps_s, scalar=1.0 / C, in1=m2,
                    op0=OP.mult, op1=OP.subtract)
                sd = pool.tile([128, cw], f32, tag=f"{tagp}_sd", name=f"{tagp}_sd")
                nc.scalar.activation(out=sd, in_=v_bc, func=AF.Sqrt,
                                     bias=eps_t, scale=1.0)
                r_bc = pool.tile([128, cw], f32, tag=f"{tagp}_rbc", name=f"{tagp}_rbc")
                nc.vector.reciprocal_approx_fast(out=r_bc, in_=sd)
                for cb in range(CT):
                    xc = pool.tile([128, cw], bf16, tag=f"{tagp}_sq", name=f"{tagp}_xc")
                    nc.vector.tensor_sub(xc, srcs[cb], m_bc)
                    nc.vector.tensor_mul(dst_fn(cb), xc, r_bc)


            def ln_fm_dual(pool, ppool, srcsA, srcsB, cw, dstA, dstB):
                """Two feature-major LNs with interleaved emission so the
                scalar/vector chains of A and B pipeline."""
                pairs = (("slnA", srcsA, dstA), ("slnB", srcsB, dstB))
                psm, pss, mbc, m2t, vbc, sdt, rbc = {}, {}, {}, {}, {}, {}, {}
                for tg, srcs, _ in pairs:
                    p = ppool.tile([128, cw], f32, tag=f"{tg}_psm", name=f"{tg}_psm")
                    for cb in range(CT):
                        nc.tensor.matmul(p, ones_f, srcs[cb],
                                         start=(cb == 0), stop=(cb == CT - 1))
                    psm[tg] = p
                for tg, srcs, _ in pairs:
                    p = ppool.tile([128, cw], f32, tag=f"{tg}_pss", name=f"{tg}_pss")
                    for cb in range(CT):
                        sq = pool.tile([128, cw], bf16, tag=f"{tg}_sq", name=f"{tg}_sq")
                        nc.scalar.activation(out=sq, in_=srcs[cb],
                                             func=AF.Square)
                        nc.tensor.matmul(p, ones_f, sq,
                                         start=(cb == 0), stop=(cb == CT - 1))
                    pss[tg] = p
                for tg, _, _ in pairs:
                    t = pool.tile([128, cw], f32, tag=f"{tg}_mbc", name=f"{tg}_mbc")
                    nc.scalar.activation(out=t, in_=psm[tg], func=AF.Copy,
                                         scale=1.0 / C)
                    mbc[tg] = t
                for tg, _, _ in pairs:
                    t = pool.tile([128, cw], bf16, tag=f"{tg}_m2", name=f"{tg}_m2")
                    nc.vector.tensor_mul(t, mbc[tg], mbc[tg])
                    m2t[tg] = t
                for tg, _, _ in pairs:
                    t = pool.tile([128, cw], f32, tag=f"{tg}_vbc", name=f"{tg}_vbc")
                    nc.vector.scalar_tensor_tensor(
                        out=t, in0=pss[tg], scalar=1.0 / C, in1=m2t[tg],
                        op0=OP.mult, op1=OP.subtract)
                    vbc[tg] = t
                for tg, _, _ in pairs:
                    t = pool.tile([128, cw], f32, tag=f"{tg}_sd", name=f"{tg}_sd")
                    nc.scalar.activation(out=t, in_=vbc[tg], func=AF.Sqrt,
                                         bias=eps_t, scale=1.0)
                    sdt[tg] = t
                for tg, _, _ in pairs:
                    t = pool.tile([128, cw], f32, tag=f"{tg}_rbc", name=f"{tg}_rbc")
                    nc.vector.reciprocal_approx_fast(out=t, in_=sdt[tg])
                    rbc[tg] = t
                for tg, srcs, dst in pairs:
                    for cb in range(CT):
                        xc = pool.tile([128, cw], bf16, tag=f"{tg}_sq", name=f"{tg}_xc")
                        nc.vector.tensor_sub(xc, srcs[cb], mbc[tg])
                        nc.vector.tensor_mul(dst(cb), xc, rbc[tg])

            # ---- persistent SBUF state ----
            with tc.tile_pool(name="apool", bufs=1) as apool:
                qh = [apool.tile([128, QPC], bf16, tag=f"qh{c}", name=f"qh{c}")
                      for c in range(CT)]
                ql = [apool.tile([128, QPC], bf16, tag=f"ql{c}", name=f"ql{c}")
                      for c in range(CT)]
                oh = [apool.tile([128, QPC], bf16, tag=f"oh{c}", name=f"oh{c}")
                      for c in range(CT)]
                ol = [apool.tile([128, QPC], bf16, tag=f"ol{c}", name=f"ol{c}")
                      for c in range(CT)]
                p1o = [apool.tile([128, QPC], bf16, tag=f"p1o{c}", name=f"p1o{c}")
                       for c in range(CT)]

                with tc.tile_pool(name="kvpool", bufs=1) as kvpool:
                    kh = [kvpool.tile([128, L], bf16, tag=f"kh{c}", name=f"kh{c}")
                          for c in range(CT)]
                    kl = [kvpool.tile([128, L], bf16, tag=f"kl{c}", name=f"kl{c}")
                          for c in range(CT)]
                    vh = [kvpool.tile([jn, C], bf16, tag=f"vh{i}", name=f"vh{i}")
                          for i, (_, jn) in enumerate(JTS)]
                    vl = [kvpool.tile([jn, C], bf16, tag=f"vl{i}", name=f"vl{i}")
                          for i, (_, jn) in enumerate(JTS)]
                    kbcol = {}
                    for a in range(2):
                        for h in range(HEADS):
                            kbcol[(a, h)] = kvpool.tile(
                                [128, NJT], f32, tag=f"kbcol{a}{h}", name=f"kbcol{a}{h}")

                    with tc.tile_pool(name="upool", bufs=1) as upool:
                        # u' = 16 * upsample(xnorm), feature-major
                        uf = [upool.tile([128, L], bf16, tag=f"uf{c}", name=f"uf{c}")
                              for c in range(CT)]
                        xnorm = [upool.tile([TT2, C], bf16, tag=f"xnorm{t}", name=f"xnorm{t}")
                                 for t in range(TOK2)]
                        p2no = [upool.tile([128, QPC], bf16, tag=f"p2no{c}", name=f"p2no{c}")
                                for c in range(CT)]
                        p1no = [upool.tile([128, QPC], bf16, tag=f"p1no{c}", name=f"p1no{c}")
                                for c in range(CT)]

                        with tc.tile_pool(name="xpool", bufs=1) as xpool:
                            xfm = [xpool.tile([128, L2], bf16, tag=f"xfm{c}", name=f"xfm{c}")
                                   for c in range(CT)]

                            # Phase 1: x = LN_core(p2 @ projT + b), token-major
                            with tc.tile_pool(name="ph1s", bufs=1) as ph1s, \
                                 tc.tile_pool(name="ph1t", bufs=3) as ph1, \
                                 tc.tile_pool(name="ph1p", bufs=2, space="PSUM") as ph1p, \
                                 tc.tile_pool(name="ph1tp", bufs=2, space="PSUM") as ph1tp:
                                ph1c = load_cat(ph1s, ph1cat, "ph1c",
                                                rows=2 * C)
                                tproj = [t[:, 0:C] for t in ph1c]
                                p2s = [t[:, C:C + L2] for t in ph1c]
                                projb_row = ph1s.tile([1, C], bf16,
                                                      tag="projb_row", name="projb_row")
                                nc.sync.dma_start(
                                    out=projb_row,
                                    in_=v_projb.rearrange("(a b) -> a b", a=1))
                                for tt in range(TOK2):
                                    ps = ph1p.tile([TT2, C], f32, tag="ps_x", name="ps_x")
                                    sl = slice(tt * TT2, (tt + 1) * TT2)
                                    for k in range(6):
                                        nc.tensor.matmul(ps, p2s[k][:, sl],
                                                         tproj[k],
                                                         start=(k == 0),
                                                         stop=False)
                                    nc.tensor.matmul(ps, ones_f[0:1, 0:TT2],
                                                     projb_row,
                                                     start=False, stop=True)
                                    st = ph1.tile([TT2, 6], f32, tag="bnst", name="bnst")
                                    nc.vector.bn_stats(out=st, in_=ps)
                                    mv = ph1.tile([TT2, 2], f32, tag="bnmv", name="bnmv")
                                    nc.vector.bn_aggr(out=mv, in_=st)
                                    sd = ph1.tile([TT2, 1], f32, tag="sd", name="sd")
                                    nc.scalar.activation(out=sd, in_=mv[:, 1:2],
                                                         func=AF.Sqrt,
                                                         bias=eps_t[0:TT2],
                                                         scale=1.0)
                                    rr = ph1.tile([TT2, 1], f32, tag="rr", name="rr")
                                    rscr = ph1.tile([TT2, 1], f32, tag="rscr", name="rscr")
                                    nc.vector.reciprocal_approx_accurate(
                                        out=rr, in_=sd, scratch=rscr)
                                    nmr = ph1.tile([TT2, 1], f32, tag="nmr", name="nmr")
                                    nc.vector.scalar_tensor_tensor(
                                        out=nmr, in0=mv[:, 0:1], scalar=-1.0,
                                        in1=rr, op0=OP.mult, op1=OP.mult)
                                    nc.scalar.activation(out=xnorm[tt], in_=ps,
                                                         func=AF.Identity,
                                                         bias=nmr, scale=rr)
                                    for cb in range(CT):
                                        pt = ph1tp.tile([128, TT2], bf16,
                                                        tag="pt", name="pt")
                                        nc.tensor.transpose(
                                            pt,
                                            xnorm[tt][:, cb * 128:(cb + 1) * 128],
                                            eye_t[0:TT2, 0:TT2])
                                        nc.vector.tensor_copy(
                                            xfm[cb][:, tt * TT2:(tt + 1) * TT2],
                                            pt)

                            # Vector bilinear upsample (x16), feature-major
                            with tc.tile_pool(name="upw", bufs=1) as upw:
                                for cb in range(CT):
                                    x3 = xfm[cb].rearrange(
                                        "p (s1 s2) -> p s1 s2", s2=H2)
                                    uat = upw.tile([128, H2 * H], bf16,
                                                   tag=f"ua{cb}", name=f"ua{cb}")
                                    a4 = uat.rearrange(
                                        "p (s1 j t) -> p s1 j t", j=H2, t=2)
                                    a3 = uat.rearrange(
                                        "p (s1 s2) -> p s1 s2", s2=H)
                                    nc.vector.scalar_tensor_tensor(
                                        out=a4[:, :, 1:, 0:1].squeeze(3),
                                        in0=x3[:, :, 1:], scalar=3.0,
                                        in1=x3[:, :, :H2 - 1],
                                        op0=OP.mult, op1=OP.add)
                                    nc.vector.scalar_tensor_tensor(
                                        out=a4[:, :, :H2 - 1, 1:2].squeeze(3),
                                        in0=x3[:, :, :H2 - 1], scalar=3.0,
                                        in1=x3[:, :, 1:],
                                        op0=OP.mult, op1=OP.add)
                                    nc.vector.tensor_scalar_mul(
                                        a4[:, :, 0:1, 0:1].squeeze(3).squeeze(2),
                                        x3[:, :, 0:1].squeeze(2), 4.0)
                                    nc.vector.tensor_scalar_mul(
                                        a4[:, :, H2 - 1:, 1:2].squeeze(3).squeeze(2),
                                        x3[:, :, H2 - 1:].squeeze(2), 4.0)
                                    u4 = uf[cb].rearrange(
                                        "p (i t s2) -> p i t s2", i=H2, t=2)
                                    nc.vector.scalar_tensor_tensor(
                                        out=u4[:, 1:, 0:1, :].squeeze(2),
                                        in0=a3[:, 1:, :], scalar=3.0,
                                        in1=a3[:, :H2 - 1, :],
                                        op0=OP.mult, op1=OP.add)
                                    nc.vector.scalar_tensor_tensor(
                                        out=u4[:, :H2 - 1, 1:2, :].squeeze(2),
                                        in0=a3[:, :H2 - 1, :], scalar=3.0,
                                        in1=a3[:, 1:, :],
                                        op0=OP.mult, op1=OP.add)
                                    nc.vector.tensor_scalar_mul(
                                        u4[:, 0:1, 0:1, :].squeeze(2).squeeze(1),
                                        a3[:, 0:1, :].squeeze(1), 4.0)
                                    nc.vector.tensor_scalar_mul(
                                        u4[:, H2 - 1:, 1:2, :].squeeze(2).squeeze(1),
                                        a3[:, H2 - 1:, :].squeeze(1), 4.0)

                        # ---- Streaming chunk loop: p1n/p2n/pp/K/V/kb, with
                        # own-quarter work interleaved as per-chunk fillers
                        with tc.tile_pool(name="sw", bufs=1) as sw, \
                             tc.tile_pool(name="stp", bufs=1) as stp, \
                             tc.tile_pool(name="stpd", bufs=2) as stpd, \
                             tc.tile_pool(name="lnt", bufs=1) as lnt, \
                             tc.tile_pool(name="stp1", bufs=1, space="PSUM") as stp1, \
                             tc.tile_pool(name="stpm", bufs=2, space="PSUM") as stpm, \
                             tc.tile_pool(name="stpv", bufs=1, space="PSUM") as stpv, \
                             tc.tile_pool(name="stpk", bufs=1, space="PSUM") as stpk:
                            wsw = load_cat(sw, wcat_sw, "wsw")
                            tkh, tkl = wslice(wsw, 0), wslice(wsw, 1)
                            tvh, tvl = wslice(wsw, 2), wslice(wsw, 3)
                            tl1L, tl1R = wslice(wsw, 4), wslice(wsw, 5)
                            tl2 = wslice(wsw, 6)
                            tqh, tql = wslice(wsw, 7), wslice(wsw, 8)
                            bl1, bl2 = bslice(bct, 3), bslice(bct, 4)
                            bqh3, bql3 = bslice(bct, 5), bslice(bct, 6)
                            bqh16, bql16 = bslice(bct16, 0), bslice(bct16, 1)
                            wupc = sw.tile([TT2, TOK2 * QPC], bf16,
                                           tag="wupc", name="wupc")
                            nc.sync.dma_start(out=wupc, in_=wupcat)
                            wupo = [wupc[:, kt * QPC:(kt + 1) * QPC]
                                    for kt in range(TOK2)]
                            for cb in range(CT):
                                nc.sync.dma_start(
                                    out=p1o[cb],
                                    in_=p1T_own[cb * 128:(cb + 1) * 128, :])

                            def fill_ownup(cb):
                                for ch in range(NQC):
                                    csl = slice(ch * QC, (ch + 1) * QC)
                                    ps = stpm.tile([128, QC], f32,
                                                   tag="ps_main", name="ps_main")
                                    for kt in range(TOK2):
                                        nc.tensor.matmul(
                                            ps,
                                            xnorm[kt][:, cb * 128:(cb + 1) * 128],
                                            wupo[kt][:, csl],
                                            start=(kt == 0),
                                            stop=(kt == TOK2 - 1))
                                    nc.scalar.activation(
                                        out=p2no[cb][:, csl], in_=ps,
                                        func=AF.Identity, bias=penb3[cb],
                                        scale=penw3[cb])

                            def fill_ownln(srcset, dst):
                                for ch, tagp in ((0, "slnA"), (1, "slnB")):
                                    csl = slice(ch * QC, (ch + 1) * QC)
                                    ln_fm_chunk(
                                        lnt, stp1,
                                        [s[:, csl] for s in srcset], QC,
                                        lambda cb, _csl=csl: dst[cb][:, _csl],
                                        tagp)

                            def fill_qproj(dst, src, tw, tb):
                                for ch in range(NQC):
                                    csl = slice(ch * QC, (ch + 1) * QC)
                                    for cb in range(CT):
                                        ps = stpm.tile([128, QC], f32,
                                                       tag="ps_main", name="ps_main")
                                        for kt in range(CT):
                                            nc.tensor.matmul(
                                                ps,
                                                tw[kt][:, cb * 128:(cb + 1) * 128],
                                                src[kt][:, csl],
                                                start=(kt == 0),
                                                stop=(kt == CT - 1))
                                        nc.scalar.activation(
                                            out=dst[cb][:, csl], in_=ps,
                                            func=AF.Identity, bias=tb[cb],
                                            scale=1.0)

                            fillers = {
                                0: lambda: fill_ownup(0),
                                1: lambda: fill_ownup(1),
                                2: lambda: (fill_ownup(2),
                                            fill_ownln(p1o, p1no)),
                                3: lambda: fill_ownln(p2no, p2no),
                                4: lambda: fill_qproj(qh, p1no, tqh, bqh3),
                                5: lambda: fill_qproj(ql, p2no, tql, bql3),
                            }

                            for ci, (c0, cw) in enumerate(KCH):
                                csl = slice(c0, c0 + cw)
                                p1ct = stpd.tile([128, CT * cw], bf16,
                                                 tag="p1ct", name="p1ct")
                                nc.sync.dma_start(
                                    out=p1ct.rearrange("p (c w) -> p c w",
                                                       c=CT),
                                    in_=p1Tt[:, :, csl])
                                p1c = [p1ct[:, cb * cw:(cb + 1) * cw]
                                       for cb in range(CT)]
                                yc = [stp.tile([128, cw], bf16, tag=f"yc{cb}", name=f"yc{cb}")
                                      for cb in range(CT)]
                                for cb in range(CT):
                                    nc.scalar.activation(
                                        out=yc[cb], in_=uf[cb][:, csl],
                                        func=AF.Identity, bias=penb3[cb],
                                        scale=penw163[cb])
                                p1nc = [stp.tile([128, cw], bf16, tag=f"p1n{cb}", name=f"p1n{cb}")
                                        for cb in range(CT)]
                                p2nc = [stp.tile([128, cw], bf16, tag=f"p2n{cb}", name=f"p2n{cb}")
                                        for cb in range(CT)]
                                ln_fm_dual(lnt, stp1, p1c, yc, cw,
                                           lambda cb: p1nc[cb],
                                           lambda cb: p2nc[cb])
                                gel = []
                                for cb in range(CT):
                                    ps = stpm.tile([128, cw], f32, tag="ps_main", name="ps_main")
                                    for kt in range(CT):
                                        nc.tensor.matmul(
                                            ps,
                                            tl1L[kt][:, cb * 128:(cb + 1) * 128],
                                            p1nc[kt], start=(kt == 0),
                                            stop=False)
                                    for kt in range(CT):
                                        nc.tensor.matmul(
                                            ps,
                                            tl1R[kt][:, cb * 128:(cb + 1) * 128],
                                            p2nc[kt], start=False,
                                            stop=(kt == CT - 1))
                                    gt = stp.tile([128, cw], bf16, tag=f"gel{cb}", name=f"gel{cb}")
                                    nc.scalar.activation(out=gt, in_=ps,
                                                         func=AF.Gelu,
                                                         bias=bl1[cb],
                                                         scale=1.0)
                                    gel.append(gt)
                                ppc = []
                                for cb in range(CT):
                                    ps = stpm.tile([128, cw], f32, tag="ps_main", name="ps_main")
                                    for kt in range(CT):
                                        nc.tensor.matmul(
                                            ps,
                                            tl2[kt][:, cb * 128:(cb + 1) * 128],
                                            gel[kt], start=(kt == 0),
                                            stop=(kt == CT - 1))
                                    ot = stp.tile([128, cw], bf16, tag=f"ppc{cb}", name=f"ppc{cb}")
                                    nc.scalar.activation(out=ot, in_=ps,
                                                         func=AF.Identity,
                                                         bias=bl2[cb],
                                                         scale=1.0)
                                    ppc.append(ot)
                                for a, (kk, vv, srcc, twk, twv, bq) in enumerate(
                                        ((kh, vh, p2nc, tkh, tvh, bqh16),
                                         (kl, vl, ppc, tkl, tvl, bql16))):
                                    for cb in range(CT):
                                        ps = stpm.tile([128, cw], f32, tag="ps_main", name="ps_main")
                                        for kt in range(CT):
                                            nc.tensor.matmul(
                                                ps,
                                                twk[kt][:, cb * 128:(cb + 1) * 128],
                                                srcc[kt], start=(kt == 0),
                                                stop=(kt == CT - 1))
                                        nc.vector.tensor_copy(
                                            kk[cb][:, csl], ps)
                                    for sub in range(max(1, cw // 128)):
                                        off = sub * 128
                                        jn = min(128, cw - off)
                                        vi = (c0 + off) // 128
                                        ps = stpv.tile([128, C], f32, tag="ps_v", name="ps_v")
                                        for kt in range(CT):
                                            nc.tensor.matmul(
                                                ps[:jn],
                                                srcc[kt][:, off:off + jn],
                                                twv[kt], start=(kt == 0),
                                                stop=(kt == CT - 1))
                                        nc.vector.tensor_copy(vv[vi],
                                                               ps[:jn])
                                    for h in range(HEADS):
                                        ps = stpk.tile([1, cw], f32, tag="ps_kb", name="ps_kb")
                                        nc.tensor.matmul(ps, bq[h],
                                                         kk[h][:, csl],
                                                         start=True, stop=True)
                                        kst = stp.tile([1, cw], f32,
                                                       tag="kbst", name="kbst")
                                        nc.vector.tensor_copy(kst, ps)
                                        nc.sync.dma_start(
                                            out=kb_d[a, h, c0:c0 + cw].rearrange(
                                                "(o n) -> o n", o=1),
                                            in_=kst)
                                if ci == len(KCH) - 1:
                                    for a in range(2):
                                        for h in range(HEADS):
                                            nc.sync.dma_start(
                                                out=kbcol[(a, h)],
                                                in_=kb_d[a, h, :].rearrange(
                                                    "(t p) -> p t", p=128))
                                if ci in fillers:
                                    fillers[ci]()

                    # ---- Attention (uf/xfm freed; K/V/q resident)
                    with tc.tile_pool(name="atw", bufs=1) as atw:
                        wat = load_cat(atw, wcat_at, "wat")
                        tfoh, tfol = wslice(wat, 0), wslice(wat, 1)
                        bfoh, bfol = bslice(bct, 7), bslice(bct, 8)
                        ones_b = atw.tile([128, 128], bf16, tag="ones_b", name="ones_b")
                        nc.vector.memset(ones_b, 1.0)
                        ones8 = atw.tile([1, 128], bf16, tag="ones8", name="ones8")
                        nc.vector.memset(ones8, 1.0)
                        tg1L, tg1R = wslice(wat, 2), wslice(wat, 3)
                        tffL, tffP = wslice(wat, 4), wslice(wat, 5)
                        tg2 = [t[:, 6 * C:6 * C + 1] for t in wat]
                        bg1, bff = bslice(bct, 9), bslice(bct, 10)
                        g2b_t = atw.tile([1, 1], f32, tag="g2b_t", name="g2b_t")
                        nc.sync.dma_start(
                            out=g2b_t, in_=v_g2b.rearrange("(a b) -> a b", a=1))

                        with tc.tile_pool(name="at", bufs=4) as at, \
                             tc.tile_pool(name="atb", bufs=2) as atb, \
                             tc.tile_pool(name="ato", bufs=1) as ato, \
                             tc.tile_pool(name="p8t", bufs=2) as p8t, \
                             tc.tile_pool(name="atps", bufs=2, space="PSUM") as atps, \
                             tc.tile_pool(name="atpo", bufs=2, space="PSUM") as atpo, \
                             tc.tile_pool(name="atpd", bufs=2, space="PSUM") as atpd, \
                             tc.tile_pool(name="atpp", bufs=2, space="PSUM") as atpp:
                            for qc in range(NQC):
                                qsl = slice(qc * QC, (qc + 1) * QC)
                                onorm = {}
                                for h in range(HEADS):
                                    ps_o = [atpo.tile([128, QC], f32, tag="ps_o", name="ps_o")
                                            for _ in range(2)]
                                    ps_d = [atpd.tile([128, QC], f32, tag="ps_d", name="ps_d")
                                            for _ in range(2)]

                                    def emit_av(pend):
                                        for (paa, pa, pi, pjn) in pend:
                                            nc.tensor.matmul(
                                                ps_o[pa],
                                                (vh if pa == 0 else vl)[pi][:, h * 128:(h + 1) * 128],
                                                paa[:pjn],
                                                start=(pi == 0),
                                                stop=(pi == NJT - 1))
                                            nc.tensor.matmul(
                                                ps_d[pa], ones_b[:pjn],
                                                paa[:pjn],
                                                start=(pi == 0),
                                                stop=(pi == NJT - 1))

                                    pend = None
                                    ebg = None
                                    for i, (j0, jn) in enumerate(JTS):
                                        if i % 5 == 0:
                                            ebg = atb.tile(
                                                [128, 5 * QC], bf16,
                                                tag="ebg", name="ebg")
                                            nc.sync.dma_start(
                                                out=ebg.rearrange(
                                                    "p (g q) -> p g q", g=5),
                                                in_=expBt[qc, h, i:i + 5].transpose(
                                                    [1, 0, 2]))
                                        eb = ebg[:, (i % 5) * QC:(i % 5 + 1) * QC]
                                        ss_pair = []
                                        for a, (kk, qq) in enumerate(
                                                ((kh, qh), (kl, ql))):
                                            ps_s = atps.tile([128, QC], f32,
                                                             tag="ps_s", name="ps_s")
                                            nc.tensor.matmul(
                                                ps_s[:jn],
                                                kk[h][:, j0:j0 + jn],
                                                qq[h][:, qsl],
                                                start=True, stop=True)
                                            ss_pair.append((ps_s, a))
                                        if pend is not None:
                                            emit_av(pend)
                                        pend = []
                                        for (ps_s, a) in ss_pair:
                                            ee = at.tile([128, QC], bf16, tag="ee", name="ee")
                                            nc.scalar.activation(
                                                out=ee[:jn], in_=ps_s[:jn],
                                                func=AF.Exp,
                                                bias=kbcol[(a, h)][:jn, i:i + 1],
                                                scale=1.0)
                                            aa = at.tile([128, QC], bf16, tag="aa", name="aa")
                                            nc.vector.tensor_mul(
                                                aa[:jn], ee[:jn], eb[:jn])
                                            pend.append((aa, a, i, jn))
                                    emit_av(pend)
                                    for a in range(2):
                                        rden = at.tile([128, QC], f32, tag="rden", name="rden")
                                        nc.vector.reciprocal_approx_fast(
                                            out=rden, in_=ps_d[a])
                                        on = ato.tile([128, QC], bf16,
                                                      tag=f"on{a}{h}", name=f"on{a}{h}")
                                        nc.vector.tensor_mul(on, ps_o[a], rden)
                                        onorm[(a, h)] = on
                                for a, (dst, tw, tb) in enumerate(
                                        ((oh, tfoh, bfoh), (ol, tfol, bfol))):
                                    for cb in range(CT):
                                        ps = atpp.tile([128, QC], f32,
                                                       tag="ps_fo", name="ps_fo")
                                        for h in range(HEADS):
                                            nc.tensor.matmul(
                                                ps,
                                                tw[h][:, cb * 128:(cb + 1) * 128],
                                                onorm[(a, h)],
                                                start=(h == 0),
                                                stop=(h == HEADS - 1))
                                        nc.scalar.activation(
                                            out=dst[cb][:, qsl], in_=ps,
                                            func=AF.Identity, bias=tb[cb],
                                            scale=1.0)
                                # phase 8 (gate, mix, ff) for this qc
                                gel = []
                                for cb in range(CT):
                                    ps = atpp.tile([128, QC], f32,
                                                   tag="ps_fo", name="ps_fo")
                                    for kt in range(CT):
                                        nc.tensor.matmul(
                                            ps,
                                            tg1L[kt][:, cb * 128:(cb + 1) * 128],
                                            oh[kt][:, qsl], start=(kt == 0),
                                            stop=False)
                                    for kt in range(CT):
                                        nc.tensor.matmul(
                                            ps,
                                            tg1R[kt][:, cb * 128:(cb + 1) * 128],
                                            ol[kt][:, qsl], start=False,
                                            stop=(kt == CT - 1))
                                    gt = p8t.tile([128, QC], bf16,
                                                  tag=f"ggel{cb}", name=f"ggel{cb}")
                                    nc.scalar.activation(out=gt, in_=ps,
                                                         func=AF.Gelu,
                                                         bias=bg1[cb],
                                                         scale=1.0)
                                    gel.append(gt)
                                ps_z = atpp.tile([1, QC], f32, tag="ps_fo", name="ps_z")
                                for kt in range(CT):
                                    nc.tensor.matmul(ps_z, tg2[kt], gel[kt],
                                                     start=(kt == 0),
                                                     stop=(kt == CT - 1))
                                gate = p8t.tile([1, QC], bf16, tag="gate", name="gate")
                                nc.scalar.activation(out=gate, in_=ps_z,
                                                     func=AF.Sigmoid,
                                                     bias=g2b_t, scale=1.0)
                                ps_gb = atpp.tile([128, QC], f32,
                                                  tag="ps_fo", name="ps_gb")
                                nc.tensor.matmul(ps_gb, ones8, gate,
                                                 start=True, stop=True)
                                mix = []
                                for cb in range(CT):
                                    dd = p8t.tile([128, QC], f32, tag="dd", name="dd")
                                    nc.vector.tensor_sub(dd, oh[cb][:, qsl],
                                                         ol[cb][:, qsl])
                                    d2 = p8t.tile([128, QC], f32, tag="d2", name="d2")
                                    nc.vector.tensor_mul(d2, dd, ps_gb)
                                    mx = p8t.tile([128, QC], bf16,
                                                  tag=f"mix{cb}", name=f"mix{cb}")
                                    nc.vector.tensor_add(mx, d2,
                                                         ol[cb][:, qsl])
                                    mix.append(mx)
                                for cb in range(CT):
                                    ps = atpp.tile([128, QC], f32,
                                                   tag="ps_fo", name="ps_ff")
                                    for kt in range(CT):
                                        nc.tensor.matmul(
                                            ps,
                                            tffL[kt][:, cb * 128:(cb + 1) * 128],
                                            mix[kt], start=(kt == 0),
                                            stop=False)
                                    for kt in range(CT):
                                        nc.tensor.matmul(
                                            ps,
                                            tffP[kt][:, cb * 128:(cb + 1) * 128],
                                            p1o[kt][:, qsl], start=False,
                                            stop=(kt == CT - 1))
                                    res = p8t.tile([128, QC], f32, tag="res", name="res")
                                    nc.scalar.activation(out=res, in_=ps,
                                                         func=AF.Identity,
                                                         bias=bff[cb],
                                                         scale=1.0)
                                    nc.sync.dma_start(
                                        out=outT[cb * 128:(cb + 1) * 128, qsl],
                                        in_=res)

    nc.compile()
    return nc


def _prepare(inputs):
    global _COMPILED
    inp = {k: np.asarray(v) for k, v in inputs.items()}
    g = _host_prep(inp)

    if _COMPILED is None:
        _COMPILED = _build()
    nc = _COMPILED

    import ml_dtypes
    bf16 = ml_dtypes.bfloat16
    p1 = inp["p1"].astype(np.float32)
    p2 = inp["p2"].astype(np.float32)
    wcat_sw = np.concatenate(
        [g["wkhT"], g["wklT"], g["wvhT"], g["wvlT"], g["pl1LT"], g["pl1RT"],
         g["pl2T"], g["wqhT"], g["wqlT"]], axis=1).astype(bf16)
    wcat_at = np.concatenate(
        [g["fohT"], g["folT"], g["g1LT"], g["g1RT"], g["ffLT"], g["ffPT"],
         g["g2T"]], axis=1).astype(bf16)
    bstack = np.stack(
        [g["penw"], g["penb"], g["penw"] / 16.0, g["pl1b"], g["pl2b"],
         g["bqh"], g["bql"], g["fohb"], g["folb"], g["g1b"], g["ffb"]])
    bcat = bstack.reshape(11, 3, 128).transpose(2, 0, 1).reshape(128, 33)
    b16 = np.stack([g["bqh"], g["bql"]]).astype(bf16)
    bcat16 = b16.reshape(2, 3, 128).transpose(2, 0, 1).reshape(128, 6)
    shared = {
        "eye128": np.eye(128, dtype=bf16),
        "v_projb": g["projb"].astype(bf16),
        "v_g2b": g["g2b"].astype(np.float32),
        "wcat_sw": wcat_sw, "wcat_at": wcat_at,
        "bcat": bcat.astype(np.float32), "bcat16": bcat16,
    }
    shared = {k: np.ascontiguousarray(v) for k, v in shared.items()}

    projT16 = g["projT"].astype(bf16)
    in_maps = []
    for core in range(NCORES):
        b, qi = divmod(core, 4)
        q0 = qi * QPC
        m = dict(shared)
        p1tc = p1[b].T.astype(bf16)                      # [C, L]
        m["p1Tt"] = np.ascontiguousarray(
            p1tc.reshape(CT, 128, L).transpose(1, 0, 2))
        m["p1T_own"] = np.ascontiguousarray(
            p1[b, q0:q0 + QPC, :].T.astype(bf16))
        m["ph1cat"] = np.ascontiguousarray(np.concatenate(
            [projT16, p2[b].T.astype(bf16)], axis=1))
        wup_own = g["WupT"][:, q0:q0 + QPC].astype(bf16)  # [784, 784]
        m["wupcat"] = np.ascontiguousarray(
            wup_own.reshape(TOK2, TT2, QPC).transpose(1, 0, 2).reshape(
                TT2, TOK2 * QPC))
        ebt = np.zeros((NQC, HEADS, NJT, 128, QC), bf16)
        for qc in range(NQC):
            for h in range(HEADS):
                for i, (j0, jn) in enumerate(JTS):
                    ebt[qc, h, i, :jn, :] = g["expB"][
                        h, j0:j0 + jn, q0 + qc * QC:q0 + (qc + 1) * QC]
        m["expBt"] = ebt
        in_maps.append(m)

    return nc, in_maps


def _run(nc, in_maps):
    from concourse.bass_utils import run_bass_kernel_spmd
    res = run_bass_kernel_spmd(nc, in_maps, core_ids=list(range(NCORES)))
    out = np.zeros((B, L, C), np.float32)
    for core in range(NCORES):
        b, qi = divmod(core, 4)
        q0 = qi * QPC
        out[b, q0:q0 + QPC, :] = res.results[core]["outT"].T
    return out


def kernel(**inputs):
    nc, in_maps = _prepare(inputs)
    return _run(nc, in_maps)
